# revision 51
# baseline (speedup 1.0000x reference)
"""Trainium2 Bass kernel for a complex-valued attention block.

Reference computation (per batch sample, complex64):
  h = ComplexGroupNorm(x)            # GN over interleaved re/im channels
  q,k,v = 1x1 complex convs of h     # channel matmuls
  attn = (q^T k) * C^-0.5            # [N,N] complex, N = H*W
  aw   = magnitude-softmax(attn)     # softmax over |attn| along keys, phase kept
  out  = v @ aw^T                    # complex
  y    = x + 1x1conv_p(out)

Sharding: 8 cores = 4 batches x 2 query-halves. Each core receives its
batch's x with the spatial axis rotated so its query block is columns
[0, 1152); k/v/GN are order-invariant in that axis so the rotation only
relabels key order inside the softmax sum.

On-chip strategy (per core):
  - attn computed TRANSPOSED [m(keys) x n(queries)] via lhsT=k, rhs=q so the
    attention weights feed the v-matmul with no transposes.
  - magnitude softmax without max-subtraction (values bounded; softmax is
    shift-invariant; SM_EPS contributes <=1e-8 relative).
  - |z| and e/|z| computed with Ln/Exp only (one ACT table set):
      l = ln(r^2+i^2); mag = exp(0.5 l); t = exp(mag - 0.5 l) = e^mag/mag
      aw_unnorm = t * (r, i);  denom = sum_m e^mag  (ones-vector matmul)
    1/denom is applied after the v-matmul (it commutes out of the m-sum).
  - matmul operands in fp16 (full-rate PE + FWL weight loads, ~11-bit
    mantissa). e^mag and t are scaled by 2^-4 inside the exp so they stay
    in fp16 range; the scale cancels exactly through the denominator.
  - phase-3 softmax elementwise work is two-stage staged per chunk (all
    Lns, then all Exps) and grouped 3 key-tiles per instruction; the ACT
    activation-table choice is steered to the single set holding ln+exp.
  - schedule: x DMAs are issued first (in 768-col strips so bn_stats can
    start on the first strip); all matmul weights arrive in ONE packed DMA
    and the small per-channel vectors in another, so the Sync engine isn't
    serializing ~45 descriptor issues in front of the input data.
  - each chunk's tail (1/denom, output projection, residual, out-DMA) is
    deferred into hook points inside the NEXT chunk's stage-A matmul loop
    so the PE never idles on the denominator chain.
"""

import os
from contextlib import ExitStack

import ml_dtypes
import numpy as np

import concourse.bacc as bacc
import concourse.bass as bass
import concourse.tile as tile
from concourse import mybir
from concourse.bass_utils import run_bass_kernel_spmd

B, C, HH, WW = 4, 256, 48, 48
N = HH * WW            # 2304 keys
NQ = N // 2            # 1152 queries per core
CK = 384               # query-chunk width
NCK = NQ // CK         # 3 chunks
MT = N // 128          # 18 key tiles
SG = 3                 # softmax elementwise group size (key tiles per op)
GN_EPS = 1e-6

F32 = mybir.dt.float32
BF16 = mybir.dt.bfloat16
FP16 = mybir.dt.float16
DT_MM = FP16          # dtype for matmul operand tiles
EXP_SHIFT = -2.772588722239781  # -4*ln2: e and t are scaled by 2^-4 to stay in fp16 range

AF = mybir.ActivationFunctionType
OP = mybir.AluOpType

# packed-weight layout: 12 names x 2 row-halves of [128, 256] fp16
W_NAMES = ("wqrT", "wqiT", "nwqiT", "wkrT", "wkiT", "nwkiT",
           "wvrT", "wviT", "nwviT", "wprT", "wpiT", "nwpiT")
W_IDX = {nm: i for i, nm in enumerate(W_NAMES)}
# packed-vector layout: 20 cols of [128,1] f32 + 16 cols of selmat
V_NAMES = ("pbr0", "pbr1", "pbi0", "pbi1", "gr0", "gr1", "gi0", "gi1",
           "br0", "br1", "bi0", "bi1", "qbr0", "qbr1", "qbi0", "qbi1",
           "kbr0", "kbr1", "kbi0", "kbi1")
V_IDX = {nm: i for i, nm in enumerate(V_NAMES)}
XSTRIP = 768           # x DMA strip width (3 bn_stats windows each)


def _emit(nc, tc, d):
    """Emit the whole SPMD program. d maps input names -> DRAM APs."""

    def pool(name, bufs, space="SBUF"):
        return tc.tile_pool(name=name, bufs=bufs, space=space)

    with pool("const", 1) as const, pool("persist", 1) as pers, \
         pool("hbuf", 1) as hbuf:
        xstack = ExitStack()
        xbuf = xstack.enter_context(pool("xbuf", 1))

        # ---- x DMAs first, in strips, so GN stats start ASAP ----
        # x arrives in fp16 (host-cast): GN stats and the h-write are its only
        # consumers and h is fp16 anyway — this halves the startup-critical
        # DMA bytes. The f32 residual path uses the separate xtr/xti tensors.
        # r strips on the sync queue, i strips on the scalar queue so the
        # two streams transfer in parallel.
        x_sb = {}
        xeng = {("r", 0): nc.sync, ("i", 0): nc.scalar,
                ("r", 1): nc.sync, ("i", 1): nc.scalar}
        for t in (0, 1):
            for p_ in ("r", "i"):
                xt = xbuf.tile([128, N], DT_MM, tag=f"x{p_}{t}", name=f"x{p_}{t}")
                src = d["xr16" if p_ == "r" else "xi16"]
                for s in range(N // XSTRIP):
                    cs = slice(s * XSTRIP, (s + 1) * XSTRIP)
                    xeng[(p_, t)].dma_start(out=xt[:, cs], in_=src[t * 128:(t + 1) * 128, cs])
                x_sb[(p_, t)] = xt

        # ---- packed constants: small GN-critical packs FIRST (vpack gates
        # the GroupNorm tail), the big weight pack last ----
        vsb = const.tile([128, 36], F32, tag="vpack", name="vpack")
        nc.sync.dma_start(out=vsb, in_=d["vpack"])

        def vop(nm):
            return vsb[:, V_IDX[nm]:V_IDX[nm] + 1]

        selmat = vsb[:, 20:36]
        rowsb = const.tile([1, 640], DT_MM, tag="rowpack", name="rowpack")
        nc.sync.dma_start(out=rowsb, in_=d["rowpack"])
        ones_row = rowsb[:, 0:128]
        bvrow = {"bvr": rowsb[:, 128:384], "bvi": rowsb[:, 384:640]}
        expmat = const.tile([16, 128], F32, tag="expmat", name="expmat")
        nc.sync.dma_start(out=expmat, in_=d["expmat"])
        ones_col = const.tile([128, 1], DT_MM, tag="ones_col", name="ones_col")
        nc.sync.dma_start(out=ones_col, in_=d["ones_col"])
        wsb = const.tile([128, 24 * 256], DT_MM, tag="wpack", name="wpack")
        nc.sync.dma_start(out=wsb, in_=d["wpack"])

        def wop(nm, half, co=None):
            base = (2 * W_IDX[nm] + half) * 256
            if co is None:
                return wsb[:, base:base + 256]
            return wsb[:, base + co * 128:base + co * 128 + 128]

        ones_row_f32 = const.tile([1, 128], F32, tag="ones_row_f32", name="ones_row_f32")
        nc.vector.memset(ones_row_f32, 1.0)
        ebias = const.tile([128, 1], F32, tag="ebias", name="ebias")
        nc.vector.memset(ebias, EXP_SHIFT)
        lbias = const.tile([128, 1], F32, tag="lbias", name="lbias")
        nc.vector.memset(lbias, 1e-35)

        # ---- persistent activations ----
        q_sb = {(p_, t): pers.tile([128, NQ], DT_MM, tag=f"q{p_}{t}", name=f"q{p_}{t}")
                for p_ in ("r", "i") for t in (0, 1)}
        k_sb = {(p_, t): pers.tile([128, N], DT_MM, tag=f"k{p_}{t}", name=f"k{p_}{t}")
                for p_ in ("r", "i") for t in (0, 1)}
        vT = {p_: pers.tile([128, MT, 256], DT_MM, tag=f"vT{p_}", name=f"vT{p_}")
              for p_ in ("r", "i")}
        nqi_sb = {co: pers.tile([128, NQ], DT_MM, tag=f"nqi{co}", name=f"nqi{co}")
                  for co in (0, 1)}
        h = {(p_, t): hbuf.tile([128, N], DT_MM, tag=f"h{p_}{t}", name=f"h{p_}{t}")
             for p_ in ("r", "i") for t in (0, 1)}

        # ================= phase 1: GroupNorm x (f32) -> h (fp16) ============
        with pool("gnw", 2) as gnw, pool("ps_small", 1, "PSUM") as ps_small, \
             nc.named_scope("groupnorm"):
            mv = {}
            acc_sums = {}

            def gn_stats(t, skip=()):
                # 4x512 + 1x256 windows (BN_STATS_FMAX=512): fewer per-op overheads
                for p_ in ("r", "i"):
                    if (p_, t) in skip:
                        continue
                    xt = x_sb[(p_, t)]
                    st = gnw.tile([128, 5, 6], F32, tag="bnstats", name="bnstats")
                    for w in range(4):
                        nc.vector.bn_stats(out=st[:, w, :], in_=xt[:, w * 512:(w + 1) * 512])
                    nc.vector.bn_stats(out=st[:, 4, :], in_=xt[:, 2048:2304])
                    m = gnw.tile([128, 2], F32, tag=f"mv{p_}{t}", name=f"mv{p_}{t}", bufs=1)
                    nc.vector.bn_aggr(out=m, in_=st)
                    mv[(p_, t)] = m

            def gn_stats_act(p_, t):
                # sum and sum-of-squares on ACT (accumulate registers), freeing DVE
                xt = x_sb[(p_, t)]
                scr = xbuf.tile([128, N], F32, tag="actscr", name="actscr", bufs=2)
                s = gnw.tile([128, 1], F32, tag=f"as{p_}{t}", name=f"as{p_}{t}", bufs=1)
                sq = gnw.tile([128, 1], F32, tag=f"aq{p_}{t}", name=f"aq{p_}{t}", bufs=1)
                nc.scalar.activation(out=scr, in_=xt, func=AF.Identity, accum_out=s)
                scr2 = xbuf.tile([128, N], F32, tag="actscr", name="actscr2", bufs=2)
                nc.scalar.activation(out=scr2, in_=xt, func=AF.Square, accum_out=sq)
                acc_sums[(p_, t)] = (s, sq)

            def gn_tail(t):
                srhs = gnw.tile([128, 4], F32, tag=f"srhs{t}", name=f"srhs{t}", bufs=1)
                for ci, p_ in enumerate(("r", "i")):
                    if (p_, t) in acc_sums:
                        s, sq = acc_sums[(p_, t)]
                        nc.vector.tensor_scalar_mul(srhs[:, 2 * ci:2 * ci + 1], s, 1.0 / N)
                        nc.vector.tensor_scalar_mul(srhs[:, 2 * ci + 1:2 * ci + 2], sq, 1.0 / N)
                        continue
                    m = mv[(p_, t)]
                    nc.vector.tensor_copy(out=srhs[:, 2 * ci:2 * ci + 1], in_=m[:, 0:1])
                    # E[x^2] = var + mean^2
                    tmp = gnw.tile([128, 1], F32, tag="gtmp", name="gtmp")
                    nc.vector.tensor_tensor(out=tmp, in0=m[:, 0:1], in1=m[:, 0:1], op=OP.mult)
                    nc.vector.tensor_tensor(out=srhs[:, 2 * ci + 1:2 * ci + 2],
                                            in0=tmp, in1=m[:, 1:2], op=OP.add)
                psg = ps_small.tile([16, 4], F32, tag="psg", name="psg")
                nc.tensor.matmul(psg, selmat, srhs, start=True, stop=True)
                gsum = gnw.tile([16, 4], F32, tag="gsum", name="gsum", bufs=1)
                nc.vector.tensor_copy(out=gsum, in_=psg)  # DVE: fewer engine hops
                st2 = gnw.tile([16, 2], F32, tag=f"st2{t}", name=f"st2{t}", bufs=1)
                mu = st2[:, 0:1]
                tmp2 = gnw.tile([16, 1], F32, tag="gtmp2", name="gtmp2")
                nc.vector.tensor_tensor(out=tmp2, in0=gsum[:, 0:1], in1=gsum[:, 2:3], op=OP.add)
                nc.vector.tensor_scalar_mul(mu, tmp2, 1.0 / 16.0)
                ex2 = gnw.tile([16, 1], F32, tag="gex2", name="gex2")
                nc.vector.tensor_tensor(out=ex2, in0=gsum[:, 1:2], in1=gsum[:, 3:4], op=OP.add)
                nc.vector.tensor_scalar_mul(ex2, ex2, 1.0 / 16.0)
                mu2 = gnw.tile([16, 1], F32, tag="gmu2", name="gmu2")
                nc.vector.tensor_tensor(out=mu2, in0=mu, in1=mu, op=OP.mult)
                var = gnw.tile([16, 1], F32, tag="gvar", name="gvar")
                nc.vector.tensor_tensor(out=var, in0=ex2, in1=mu2, op=OP.subtract)
                # rstd = exp(-0.5 * ln(var + eps))
                epst = gnw.tile([16, 1], F32, tag="geps", name="geps", bufs=1)
                nc.vector.memset(epst, GN_EPS)
                lnv = gnw.tile([16, 1], F32, tag="glnv", name="glnv")
                nc.scalar.activation(out=lnv, in_=var, func=AF.Ln, bias=epst, scale=1.0)
                nc.scalar.activation(out=st2[:, 1:2], in_=lnv, func=AF.Exp, bias=0.0, scale=-0.5)
                psr = ps_small.tile([128, 2], F32, tag="psr", name="psr")
                nc.tensor.matmul(psr, expmat, st2, start=True, stop=True)
                gst = gnw.tile([128, 2], F32, tag=f"gst{t}", name=f"gst{t}", bufs=1)
                nc.vector.tensor_copy(out=gst, in_=psr)  # DVE: fewer engine hops
                for p_ in ("r", "i"):
                    gam = vop(f"g{p_}{t}")
                    bet = vop(f"b{p_}{t}")
                    sc = gnw.tile([128, 1], F32, tag=f"sc{p_}{t}", name=f"sc{p_}{t}", bufs=1)
                    nc.vector.tensor_tensor(out=sc, in0=gst[:, 1:2], in1=gam, op=OP.mult)
                    bi = gnw.tile([128, 1], F32, tag=f"bi{p_}{t}", name=f"bi{p_}{t}", bufs=1)
                    nc.vector.tensor_tensor(out=bi, in0=gst[:, 0:1], in1=sc, op=OP.mult)
                    nc.vector.scalar_tensor_tensor(out=bi, in0=bi, scalar=-1.0,
                                                   in1=bet, op0=OP.mult, op1=OP.add)
                    # h in column halves so q/k projections start earlier; real
                    # part on ACT, imag on DVE (t=1) / GpSimd (t=0) in parallel
                    for hv in (0, 1):
                        cs = slice(hv * (N // 2), (hv + 1) * (N // 2))
                        if p_ == "i":
                            nc.vector.tensor_scalar(out=h[(p_, t)][:, cs], in0=x_sb[(p_, t)][:, cs],
                                                    scalar1=sc, scalar2=bi, op0=OP.mult, op1=OP.add)
                        else:
                            nc.scalar.activation(out=h[(p_, t)][:, cs], in_=x_sb[(p_, t)][:, cs],
                                                 func=AF.Identity, bias=bi, scale=sc)

            # last tile's stats on ACT accumulate-registers (nothing else in
            # the ACT queue ahead of it now); DVE handles the other three
            gn_stats_act("i", 1)
            gn_stats(0)
            gn_tail(0)
            gn_stats(1, skip=(("i", 1),))
            gn_tail(1)
        xstack.close()  # x tiles dead after GN; free their SBUF for phase 3

        # v-bias broadcast, emitted after GN so its ACT copy never blocks the
        # GroupNorm-critical ACT queue (first consumer is emit_vt, much later)
        bv_bc = {}
        with tc.tile_pool(name="ps_bv", bufs=1, space="PSUM") as ps_bv:
            for nm in ("bvr", "bvi"):
                psb = ps_bv.tile([128, 256], F32, tag="psb", name="psb")
                nc.tensor.matmul(psb, ones_row, bvrow[nm], start=True, stop=True)
                t = const.tile([128, 256], F32, tag=nm + "_bc", name=nm + "_bc")
                nc.scalar.copy(out=t, in_=psb)
                bv_bc[nm] = t

        # ================= phase 2: q/k projections =================
        with pool("ps12", 6, "PSUM") as ps12, nc.named_scope("qkv"):

            def proj_qk(dst, wa, wb, bias_ap, cols, co, dve_evac=False):
                ps = ps12.tile([128, CK], F32, tag="ps12", name="ps_qk")
                # accumulation order r0,i0,r1,i1: first two only need t=0
                nc.tensor.matmul(ps, wop(wa, 0, co), h[("r", 0)][:, cols], start=True, stop=False)
                nc.tensor.matmul(ps, wop(wb, 0, co), h[("i", 0)][:, cols], start=False, stop=False)
                nc.tensor.matmul(ps, wop(wa, 1, co), h[("r", 1)][:, cols], start=False, stop=False)
                nc.tensor.matmul(ps, wop(wb, 1, co), h[("i", 1)][:, cols], start=False, stop=True)
                if dve_evac:
                    nc.vector.tensor_scalar_add(dst[:, cols], ps, bias_ap)
                else:
                    nc.scalar.activation(out=dst[:, cols], in_=ps, func=AF.Identity,
                                         bias=bias_ap, scale=1.0)

            for ic in range(NCK):
                cols = slice(ic * CK, (ic + 1) * CK)
                for co in (0, 1):
                    proj_qk(q_sb[("r", co)], "wqrT", "nwqiT", vop(f"qbr{co}"), cols, co)
                    proj_qk(q_sb[("i", co)], "wqiT", "wqrT", vop(f"qbi{co}"), cols, co)
            for ic in range(N // CK):
                cols = slice(ic * CK, (ic + 1) * CK)
                for co in (0, 1):
                    proj_qk(k_sb[("r", co)], "wkrT", "nwkiT", vop(f"kbr{co}"), cols, co, dve_evac=True)
                    proj_qk(k_sb[("i", co)], "wkiT", "wkrT", vop(f"kbi{co}"), cols, co, dve_evac=True)

        # ================= phase 3: attention =================
        # Per chunk: stage A stages attn^T (r,i) in fp16 + lf = ln(r^2+i^2);
        # stage B turns them into aw via exps and accumulates v @ aw.
        # The tail of chunk ic (1/den, p-proj, residual, out-DMA) is emitted
        # via hooks inside chunk ic+1's stage-A loop so the PE never stalls.
        with pool("stage", 1) as stg, pool("sm", 2) as smp, pool("ck", 1) as ckp, \
             pool("ps_attn", 3, "PSUM") as psA, pool("ps_den", 1, "PSUM") as psD, \
             pool("ps_mm", 4, "PSUM") as psM:

            def emit_vt():
                # vT[m, co] = sum_ci h[ci, m] WvT[ci, co] + bv
                for j in range(MT):
                    msl = slice(j * 128, (j + 1) * 128)
                    for p_, wa, wb, brow in (("r", "wvrT", "nwviT", "bvr"),
                                             ("i", "wviT", "wvrT", "bvi")):
                        ps = psM.tile([128, 256], F32, tag="mm_ps", name="ps_vt")
                        nc.tensor.matmul(ps, h[("r", 0)][:, msl], wop(wa, 0), start=True, stop=False, skip_group_check=True)
                        nc.tensor.matmul(ps, h[("i", 0)][:, msl], wop(wb, 0), start=False, stop=False, skip_group_check=True)
                        nc.tensor.matmul(ps, h[("r", 1)][:, msl], wop(wa, 1), start=False, stop=False, skip_group_check=True)
                        nc.tensor.matmul(ps, h[("i", 1)][:, msl], wop(wb, 1), start=False, stop=True, skip_group_check=True)
                        nc.vector.tensor_tensor(out=vT[p_][:, j, :], in0=ps, in1=bv_bc[brow], op=OP.add)

            for co in (0, 1):
                nc.vector.tensor_scalar_mul(nqi_sb[co], q_sb[("i", co)], -1.0)

            def stage_a(ic, hooks):
                cols = slice(ic * CK, (ic + 1) * CK)
                nqi = {co: nqi_sb[co][:, cols] for co in (0, 1)}
                cpr = stg.tile([128, MT, CK], DT_MM, tag="cpr", name="cpr")
                cpi = stg.tile([128, MT, CK], DT_MM, tag="cpi", name="cpi")
                lf = stg.tile([128, MT, CK], F32, tag="lf", name="lf")
                for j in range(MT):
                    msl = slice(j * 128, (j + 1) * 128)
                    ps_ar = psA.tile([128, CK], F32, tag="attn_ps", name="ps_ar")
                    ps_ai = psA.tile([128, CK], F32, tag="attn_ps", name="ps_ai")
                    # same-lhsT matmuls back to back (kr0 kr1 ki0 ki1 each used twice)
                    nc.tensor.matmul(ps_ar, k_sb[("r", 0)][:, msl], q_sb[("r", 0)][:, cols], start=True, stop=False, skip_group_check=True)
                    nc.tensor.matmul(ps_ai, k_sb[("r", 0)][:, msl], q_sb[("i", 0)][:, cols], start=True, stop=False, skip_group_check=True)
                    nc.tensor.matmul(ps_ar, k_sb[("r", 1)][:, msl], q_sb[("r", 1)][:, cols], start=False, stop=False, skip_group_check=True)
                    nc.tensor.matmul(ps_ai, k_sb[("r", 1)][:, msl], q_sb[("i", 1)][:, cols], start=False, stop=False, skip_group_check=True)
                    nc.tensor.matmul(ps_ar, k_sb[("i", 0)][:, msl], nqi[0], start=False, stop=False, skip_group_check=True)
                    nc.tensor.matmul(ps_ai, k_sb[("i", 0)][:, msl], q_sb[("r", 0)][:, cols], start=False, stop=False, skip_group_check=True)
                    nc.tensor.matmul(ps_ar, k_sb[("i", 1)][:, msl], nqi[1], start=False, stop=True, skip_group_check=True)
                    nc.tensor.matmul(ps_ai, k_sb[("i", 1)][:, msl], q_sb[("r", 1)][:, cols], start=False, stop=True, skip_group_check=True)

                    nc.scalar.copy(out=cpr[:, j, :], in_=ps_ar)
                    if j % 3 == 0:
                        # DVE paces stage A; offload a third of the cpi evacs to ACT
                        nc.scalar.copy(out=cpi[:, j, :], in_=ps_ai)
                    else:
                        nc.vector.tensor_copy(out=cpi[:, j, :], in_=ps_ai)
                    if j % SG == SG - 1:
                        g = slice(j - SG + 1, j + 1)
                        r2 = smp.tile([128, SG, CK], DT_MM, tag="r2", name="r2")
                        i2 = smp.tile([128, SG, CK], DT_MM, tag="i2", name="i2")
                        m2 = smp.tile([128, SG, CK], DT_MM, tag="m2", name="m2")
                        nc.vector.tensor_tensor(out=r2, in0=cpr[:, g, :], in1=cpr[:, g, :], op=OP.mult)
                        nc.vector.tensor_tensor(out=i2, in0=cpi[:, g, :], in1=cpi[:, g, :], op=OP.mult)
                        nc.vector.tensor_tensor(out=m2, in0=r2, in1=i2, op=OP.add)
                        nc.scalar.activation(out=lf[:, g, :], in_=m2, func=AF.Ln, bias=lbias)
                    for fn in hooks.get(j, ()):
                        fn()
                return cols, cpr, cpi, lf

            def stage_b(ic, cols, cpr, cpi, lf, den_early=False):
                ps_den = psD.tile([1, CK], F32, tag="den", name="den")
                tl = {"ic": ic, "cols": cols, "ps_den": ps_den}
                accs = tl["accs"] = {}
                for key in (("r", 0), ("r", 1), ("i", 0), ("i", 1)):
                    accs[key] = psM.tile([128, CK], F32, tag="mm_ps", name=f"acc{key[0]}{key[1]}")
                for jg in range(MT // SG):
                    g = slice(jg * SG, (jg + 1) * SG)
                    mag = smp.tile([128, SG, CK], F32, tag="mag", name="mag")
                    nc.scalar.activation(out=mag, in_=lf[:, g, :], func=AF.Exp, scale=0.5)
                    u = smp.tile([128, SG, CK], F32, tag="u", name="u")
                    nc.vector.scalar_tensor_tensor(out=u, in0=lf[:, g, :], scalar=-0.5,
                                                   in1=mag, op0=OP.mult, op1=OP.add)
                    tt = smp.tile([128, SG, CK], DT_MM, tag="tt", name="tt")
                    nc.scalar.activation(out=tt, in_=u, func=AF.Exp, bias=ebias)   # t/16
                    te = smp.tile([128, SG, CK], DT_MM, tag="te", name="te")
                    nc.scalar.activation(out=te, in_=mag, func=AF.Exp, bias=ebias)  # e/16
                    nc.vector.tensor_tensor(out=cpr[:, g, :], in0=tt, in1=cpr[:, g, :], op=OP.mult)
                    nc.vector.tensor_tensor(out=cpi[:, g, :], in0=tt, in1=cpi[:, g, :], op=OP.mult)
                    naw = smp.tile([128, SG, CK], DT_MM, tag="nawi", name="nawi", bufs=2)
                    nc.vector.tensor_scalar_mul(naw, cpi[:, g, :], -1.0)
                    # den matmuls grouped: same ones_col lhsT three times in a row
                    for js in range(SG):
                        j = jg * SG + js
                        nc.tensor.matmul(ps_den, ones_col[:], te[:, js, :],
                                         start=(j == 0), stop=(j == MT - 1), skip_group_check=True)
                    if den_early and jg == MT // SG - 1:
                        # last chunk: 1/den chain overlaps the final v-matmul group
                        den_bcast(tl)
                    for js in range(SG):
                        j = jg * SG + js
                        st_ = (j == 0)
                        sp_ = (j == MT - 1)
                        # vT_r used twice back to back; then vT_i twice
                        nc.tensor.matmul(accs[("r", 0)], vT["r"][:, j, 0:128], cpr[:, j, :], start=st_, stop=sp_, skip_group_check=True)
                        nc.tensor.matmul(accs[("i", 0)], vT["r"][:, j, 0:128], cpi[:, j, :], start=st_, stop=sp_, skip_group_check=True)
                        nc.tensor.matmul(accs[("r", 1)], vT["r"][:, j, 128:256], cpr[:, j, :], start=st_, stop=sp_, skip_group_check=True)
                        nc.tensor.matmul(accs[("i", 1)], vT["r"][:, j, 128:256], cpi[:, j, :], start=st_, stop=sp_, skip_group_check=True)
                        nc.tensor.matmul(accs[("i", 0)], vT["i"][:, j, 0:128], cpr[:, j, :], start=False, stop=False, skip_group_check=True)
                        nc.tensor.matmul(accs[("r", 0)], vT["i"][:, j, 0:128], naw[:, js, :], start=False, stop=False, skip_group_check=True)
                        nc.tensor.matmul(accs[("i", 1)], vT["i"][:, j, 128:256], cpr[:, j, :], start=False, stop=False, skip_group_check=True)
                        nc.tensor.matmul(accs[("r", 1)], vT["i"][:, j, 128:256], naw[:, js, :], start=False, stop=False, skip_group_check=True)
                return tl

            def den_bcast(tl):
                den_row = ckp.tile([1, CK], F32, tag="den_row", name="den_row", bufs=2)
                nc.scalar.copy(out=den_row, in_=tl["ps_den"])
                rec_row = ckp.tile([1, CK], F32, tag="rec_row", name="rec_row", bufs=2)
                nc.vector.reciprocal_approx_fast(out=rec_row, in_=den_row)
                ps_rep = psD.tile([128, CK], F32, tag="den", name="ps_rep")
                nc.tensor.matmul(ps_rep, ones_row_f32[:], rec_row[:], start=True, stop=True, skip_group_check=True)
                tl["sden"] = ckp.tile([128, CK], F32, tag="sden", name="sden", bufs=1)
                nc.vector.tensor_copy(out=tl["sden"], in_=ps_rep)

            def outsc_one(tl, key, eng=None):
                t = ckp.tile([128, CK], DT_MM, tag=f"outsc{key[0]}{key[1]}",
                             name=f"outsc{key[0]}{key[1]}", bufs=1)
                (eng or nc.vector).tensor_tensor(out=t, in0=tl["accs"][key], in1=tl["sden"], op=OP.mult)
                tl.setdefault("outsc", {})[key] = t

            def tail_rest(tl):
                ic, outsc = tl["ic"], tl["outsc"]
                with nc.named_scope(f"tail{ic}"):
                    for p_, wa, wb in (("r", "wprT", "nwpiT"), ("i", "wpiT", "wprT")):
                        for co in (0, 1):
                            ps = psM.tile([128, CK], F32, tag="mm_ps", name="ps_proj")
                            nc.tensor.matmul(ps, wop(wa, 0, co), outsc[("r", 0)][:], start=True, stop=False, skip_group_check=True)
                            nc.tensor.matmul(ps, wop(wa, 1, co), outsc[("r", 1)][:], start=False, stop=False, skip_group_check=True)
                            nc.tensor.matmul(ps, wop(wb, 0, co), outsc[("i", 0)][:], start=False, stop=False, skip_group_check=True)
                            nc.tensor.matmul(ps, wop(wb, 1, co), outsc[("i", 1)][:], start=False, stop=True, skip_group_check=True)
                            idx = co * NCK + ic
                            rs = slice(idx * 128, (idx + 1) * 128)
                            xres = ckp.tile([128, CK], F32, tag=f"xres{p_}{co}", name=f"xres{p_}{co}", bufs=1)
                            nc.sync.dma_start(out=xres, in_=d["xtr" if p_ == "r" else "xti"][rs, :])
                            nc.vector.scalar_tensor_tensor(out=xres, in0=ps, scalar=vop(f"pb{p_}{co}"),
                                                           in1=xres, op0=OP.add, op1=OP.add)
                            nc.sync.dma_start(out=d["outr" if p_ == "r" else "outi"][rs, :], in_=xres)

            prev = None
            for ic in range(NCK):
                hooks = {}
                if prev is not None:
                    tl = prev
                    hooks[5] = (lambda tl=tl: den_bcast(tl),)
                    hooks[8] = (lambda tl=tl: outsc_one(tl, ("r", 0)),)
                    hooks[10] = (lambda tl=tl: outsc_one(tl, ("r", 1)),)
                    hooks[12] = (lambda tl=tl: outsc_one(tl, ("i", 0)),)
                    hooks[14] = (lambda tl=tl: outsc_one(tl, ("i", 1)),)
                with nc.named_scope(f"attn_chunk{ic}"):
                    cols, cpr, cpi, lf = stage_a(ic, hooks)
                    if ic == 0:
                        emit_vt()
                    if prev is not None:
                        tail_rest(prev)
                    prev = stage_b(ic, cols, cpr, cpi, lf, den_early=(ic == NCK - 1))
            # final tail is exposed: interleave outsc with the projection
            # matmuls. Group g starts accumulating at emission step g (the
            # moment its psum slot frees) and wraps through all four parts,
            # so the PE never waits on a WAR dependency.
            with nc.named_scope("tail_final"):
                ic = prev["ic"]
                parts = (("r", 0), ("r", 1), ("i", 0), ("i", 1))
                # prefetch the residual tiles: off the tail's dependency chain
                xres_t = {}
                for p_, co in parts:
                    idx = co * NCK + ic
                    rs = slice(idx * 128, (idx + 1) * 128)
                    xres = ckp.tile([128, CK], F32, tag=f"xres{p_}{co}", name=f"xres{p_}{co}", bufs=1)
                    nc.sync.dma_start(out=xres, in_=d["xtr" if p_ == "r" else "xti"][rs, :])
                    xres_t[(p_, co)] = xres
                fps = {}
                for s in range(7):
                    if s < 4:
                        outsc_one(prev, parts[s])
                    part = parts[s % 4]
                    o = prev["outsc"][part]
                    pr, half = part
                    for g, (p_, co) in enumerate(parts):
                        if not (g <= s <= g + 3):
                            continue
                        if s == g:
                            fps[(p_, co)] = psM.tile([128, CK], F32, tag="mm_ps", name=f"ps_f{p_}{co}")
                        wnm = (("wprT" if pr == "r" else "nwpiT") if p_ == "r"
                               else ("wpiT" if pr == "r" else "wprT"))
                        nc.tensor.matmul(fps[(p_, co)], wop(wnm, half, co), o[:],
                                         start=(s == g), stop=(s == g + 3),
                                         skip_group_check=True)
                        if s == g + 3:
                            idx = co * NCK + ic
                            rs = slice(idx * 128, (idx + 1) * 128)
                            xres = xres_t[(p_, co)]
                            nc.vector.scalar_tensor_tensor(out=xres, in0=fps[(p_, co)], scalar=vop(f"pb{p_}{co}"),
                                                           in1=xres, op0=OP.add, op1=OP.add)
                            # alternate output DMAs across the two queues
                            oeng = nc.sync if p_ == "r" else nc.scalar
                            oeng.dma_start(out=d["outr" if p_ == "r" else "outi"][rs, :], in_=xres)


_CACHE = {}


def _build():
    if "nc" in _CACHE:
        return _CACHE["nc"]
    nc = bacc.Bacc("TRN2", target_bir_lowering=False, debug=False, num_devices=8)
    d = {}
    ins = {
        "xr16": ((C, N), DT_MM), "xi16": ((C, N), DT_MM),
        "xtr": ((2 * NCK * 128, CK), F32), "xti": ((2 * NCK * 128, CK), F32),
        "wpack": ((128, 24 * 256), DT_MM),
        "vpack": ((128, 36), F32),
        "rowpack": ((1, 640), DT_MM),
        "expmat": ((16, 128), F32),
        "ones_col": ((128, 1), DT_MM),
    }
    for nm, (sh, dt_) in ins.items():
        d[nm] = nc.dram_tensor(nm, list(sh), dt_, kind="ExternalInput").ap()
    for nm in ("outr", "outi"):
        d[nm] = nc.dram_tensor(nm, [2 * NCK * 128, CK], F32, kind="ExternalOutput").ap()
    with tile.TileContext(nc) as tc:
        _emit(nc, tc, d)
    # Steer walrus's per-function activation-table choice to the one set that
    # holds BOTH ln and exp, so the softmax pipeline never reloads tables.
    import concourse.bacc as _bacc_mod
    _orig_tables = _bacc_mod.get_activation_tables

    def _tables_ln_exp_combined(arch):
        tabs = _orig_tables(arch)
        ln_exp = {mybir.ActivationFunctionType.Ln, mybir.ActivationFunctionType.Exp}
        return {
            name: (fns if name == "natural_log_exp_and_others" else (fns - ln_exp))
            for name, fns in tabs.items()
        }

    _bacc_mod.get_activation_tables = _tables_ln_exp_combined
    try:
        nc.compile()
    finally:
        _bacc_mod.get_activation_tables = _orig_tables
    _CACHE["nc"] = nc
    return nc


def _mm_cast(a):
    """Convert fp32 ndarray to the matmul operand dtype (fp16)."""
    return np.ascontiguousarray(np.asarray(a, np.float32).astype(np.float16))


def kernel(x_ri, gn_gamma, gn_beta, qw, qb, kw, kb, vw, vb, pw, pb):
    x_ri = np.asarray(x_ri, np.float32)
    s = float(C) ** -0.5
    qw, qb = np.asarray(qw, np.float32), np.asarray(qb, np.float32)
    kw, kb = np.asarray(kw, np.float32), np.asarray(kb, np.float32)
    vw, vb = np.asarray(vw, np.float32), np.asarray(vb, np.float32)
    pw, pb = np.asarray(pw, np.float32), np.asarray(pb, np.float32)
    gn_gamma = np.asarray(gn_gamma, np.float32)
    gn_beta = np.asarray(gn_beta, np.float32)

    wmats = {
        "wqrT": qw[0].T * s, "wqiT": qw[1].T * s, "nwqiT": -qw[1].T * s,
        "wkrT": kw[0].T, "wkiT": kw[1].T, "nwkiT": -kw[1].T,
        "wvrT": vw[0].T, "wviT": vw[1].T, "nwviT": -vw[1].T,
        "wprT": pw[0].T, "wpiT": pw[1].T, "nwpiT": -pw[1].T,
    }
    wpack = np.empty((128, 24 * 256), np.float16)
    for nm, arr in wmats.items():
        a16 = _mm_cast(arr)
        for half in (0, 1):
            base = (2 * W_IDX[nm] + half) * 256
            wpack[:, base:base + 256] = a16[half * 128:(half + 1) * 128, :]

    vcols = {
        "pbr0": pb[0][:128], "pbr1": pb[0][128:], "pbi0": pb[1][:128], "pbi1": pb[1][128:],
        "gr0": gn_gamma[0::2][:128], "gr1": gn_gamma[0::2][128:],
        "gi0": gn_gamma[1::2][:128], "gi1": gn_gamma[1::2][128:],
        "br0": gn_beta[0::2][:128], "br1": gn_beta[0::2][128:],
        "bi0": gn_beta[1::2][:128], "bi1": gn_beta[1::2][128:],
        "qbr0": qb[0][:128] * s, "qbr1": qb[0][128:] * s,
        "qbi0": qb[1][:128] * s, "qbi1": qb[1][128:] * s,
        "kbr0": kb[0][:128], "kbr1": kb[0][128:], "kbi0": kb[1][:128], "kbi1": kb[1][128:],
    }
    vpack = np.empty((128, 36), np.float32)
    for nm, col in vcols.items():
        vpack[:, V_IDX[nm]] = np.asarray(col, np.float32)
    vpack[:, 20:36] = np.eye(16, dtype=np.float32)[np.arange(128) // 8]

    rowpack = np.empty((1, 640), np.float16)
    rowpack[0, 0:128] = 1.0
    rowpack[0, 128:384] = vb[0].astype(np.float16)
    rowpack[0, 384:640] = vb[1].astype(np.float16)

    common = {
        "wpack": wpack,
        "vpack": vpack,
        "rowpack": rowpack,
        "expmat": np.ascontiguousarray(np.eye(16, dtype=np.float32)[np.arange(128) // 8].T),
        "ones_col": np.ones((128, 1), np.float16),
    }

    xr = np.ascontiguousarray(x_ri[..., 0].reshape(B, C, N))
    xi = np.ascontiguousarray(x_ri[..., 1].reshape(B, C, N))

    def retile(a):
        # [256, 1152] -> [768, 384] blocks: row idx = co*NCK+ic holds
        # a[co*128:(co+1)*128, ic*CK:(ic+1)*CK]
        return np.ascontiguousarray(
            a[:, :NQ].reshape(2, 128, NCK, CK).transpose(0, 2, 1, 3).reshape(2 * NCK * 128, CK))

    in_maps = []
    for core in range(8):
        b, half = core // 2, core % 2
        q0 = half * NQ
        xrb = np.ascontiguousarray(np.roll(xr[b], -q0, axis=1))
        xib = np.ascontiguousarray(np.roll(xi[b], -q0, axis=1))
        in_maps.append({
            **common,
            "xr16": xrb.astype(np.float16), "xi16": xib.astype(np.float16),
            "xtr": retile(xrb), "xti": retile(xib),
        })

    nc = _build()
    trace = os.environ.get("BASS_KERNEL_TRACE") == "1"
    res = run_bass_kernel_spmd(nc, in_maps, core_ids=list(range(8)), trace=trace)
    kernel._last_result = res

    def untile(a):
        return a.reshape(2, NCK, 128, CK).transpose(0, 2, 1, 3).reshape(C, NQ)

    out = np.empty((B, C, N), np.complex64)
    for core in range(8):
        b, half = core // 2, core % 2
        q0 = half * NQ
        rr = res.results[core]
        out[b, :, q0:q0 + NQ] = untile(rr["outr"]) + 1j * untile(rr["outi"])
    return out.reshape(B, C, HH, WW)


kernel._last_result = None
